# revision 28
# baseline (speedup 1.0000x reference)
"""Distributed Bass kernel for nn_Attention (B=2, T=2048, D=1024, H=16) on 8 TRN2 cores.

Sharding: core c -> (batch b = c//4, head-group g = c%4, heads 4g..4g+3).
QKV tensor-parallel over heads, out-proj row-parallel + ReduceScatter(4-rank groups).

v2: host-normalized weights, K=64 row-tiled score matmuls (two heads
concurrently on PE row-group halves), norms folded into q/k so the softmax
exp is a single unscaled [128,1024] ACT instruction per (head-pair, j),
bf16 output, chunked ReduceScatter overlapped with attention.
"""

import functools
import numpy as np
from contextlib import ExitStack

B, T, D, H, HD = 2, 2048, 1024, 16, 64
EPS = 1e-4
NCORES, GROUP = 8, 4
HL = H // GROUP          # heads per core = 4
DL = HL * HD             # local feature cols = 256
NTT = T // 128           # 16 token tiles
NDT = D // 128           # 8 d tiles
WCOLS = 3 * DL           # 768 qkv output cols per core

# attention q-blocks (tok0, width) and reduce-scatter chunks (tok0, width)
BLOCKS = [(0, 512), (512, 512), (1024, 512), (1536, 512)]
CHUNKS = [(0, 512), (512, 512), (1024, 512), (1536, 512)]
# chunk ids queued after each block completes
BLOCK_CHUNKS = [[0], [1], [2], [3]]


def _build_bass():
    import concourse.bass as bass
    import concourse.tile as tile
    from concourse import bacc, mybir

    f32 = mybir.dt.float32
    f32r = mybir.dt.float32r
    bf16 = mybir.dt.bfloat16
    AX = mybir.AxisListType
    OP = mybir.AluOpType
    AF = mybir.ActivationFunctionType

    nc = bacc.Bacc("TRN2", target_bir_lowering=False, debug=False, num_devices=NCORES)

    xT_ext = nc.dram_tensor("xT", [D, T], bf16, kind="ExternalInput").ap()
    whT_ext = nc.dram_tensor("whT", [D, WCOLS], bf16, kind="ExternalInput").ap()
    WT_ext = nc.dram_tensor("WT", [DL, D], bf16, kind="ExternalInput").ap()
    out_ext = nc.dram_tensor("out", [DL, T], bf16, kind="ExternalOutput").ap()

    ones_np = np.ones((1, 64), dtype=np.float32)

    with tile.TileContext(nc) as tc, ExitStack() as ctx:
        # ---------------- persistent pools ----------------
        pers = ctx.enter_context(tc.tile_pool(name="pers", bufs=1))
        dram = ctx.enter_context(tc.tile_pool(name="dram", bufs=1, space="DRAM"))

        ones_sb = pers.tile([1, 64], f32r)
        ones_dram = nc.inline_tensor(ones_np, name="ones_c")
        nc.gpsimd.dma_start(ones_sb[:], ones_dram.ap())

        xT_sb = pers.tile([128, NDT, T], bf16)
        whT_sb = pers.tile([128, NDT, WCOLS], bf16)
        WT_sb = pers.tile([128, 2, D], bf16)
        qT_sb = [pers.tile([128, T], bf16, name=f"qT{rb}") for rb in range(2)]
        kT_sb = [pers.tile([128, T], bf16, name=f"kT{rb}") for rb in range(2)]
        v_sb = pers.tile([128, NTT, HL, 65], bf16)
        aoT_sb = [pers.tile([128, T], bf16, name=f"aoT{rb}") for rb in range(2)]

        qknat = dram.tile([T, 2 * DL], bf16)
        rs_in = [dram.tile([D, w], bf16, name=f"rs_in{k}")
                 for k, (_, w) in enumerate(CHUNKS)]
        rs_out = [dram.tile([DL, w], bf16, name=f"rs_out{k}")
                  for k, (_, w) in enumerate(CHUNKS)]


        # ---------------- input DMAs (WT first: small, feeds PE warm-up) ----
        nc.sync.dma_start(WT_sb[:], WT_ext.rearrange("(n p) d -> p n d", p=128))
        nc.sync.dma_start(whT_sb[:], whT_ext.rearrange("(n p) c -> p n c", p=128))
        # x in 4 token chunks so QKV can start after the first lands
        for xc in range(4):
            nc.gpsimd.dma_start(
                xT_sb[:, :, 512 * xc : 512 * (xc + 1)],
                xT_ext.rearrange("(n p) t -> p n t", p=128)[
                    :, :, 512 * xc : 512 * (xc + 1)],
            )
        # ones column for the fused av-rowsum matmul
        nc.vector.memset(v_sb[:, :, :, 64:65], 1.0)

        # ---------------- QKV phase ----------------
        with tc.tile_pool(name="qps", bufs=2, space="PSUM") as qps, \
             tc.tile_pool(name="qsb", bufs=3) as qsb, \
             tc.tile_pool(name="wps", bufs=2, space="PSUM") as wps:
            # PE warm-up: dependency-free matmuls while x/whT stream in
            for wu in range(14):
                wt = wps.tile([128, 512], f32, name="wt", tag="warm")
                nc.tensor.matmul(wt[:], WT_sb[:, 0, 0:128], WT_sb[:, 0, 0:512],
                                 start=True, stop=True)

            for tt in range(NTT):
                ps = qps.tile([128, WCOLS], f32)
                for dt_ in range(NDT):
                    lhsT = xT_sb[:, dt_, 128 * tt : 128 * (tt + 1)]
                    nc.tensor.matmul(ps[:, 0:512], lhsT, whT_sb[:, dt_, 0:512],
                                     start=(dt_ == 0), stop=(dt_ == NDT - 1))
                    nc.tensor.matmul(ps[:, 512:768], lhsT, whT_sb[:, dt_, 512:768],
                                     start=(dt_ == 0), stop=(dt_ == NDT - 1))
                # evict q+k raw and v quickly so ps recycles (ACT, 2 instrs)
                qk_sb = qsb.tile([128, 2 * DL], bf16, name="qk_sb")
                nc.scalar.activation(qk_sb[:], ps[:, 0 : 2 * DL], AF.Copy)
                nc.scalar.activation(
                    v_sb[:, tt, :, 0:64],
                    ps[:, 2 * DL : 3 * DL].rearrange("p (h c) -> p h c", c=HD),
                    AF.Copy)
                # per-head norms of q and k off the SBUF copy
                sq = qsb.tile([128, 2 * DL], bf16, name="sq")
                nc.vector.tensor_tensor(sq[:], qk_sb[:], qk_sb[:], op=OP.mult)
                ns = qsb.tile([128, 2 * HL], f32, name="ns")
                nc.vector.reduce_sum(
                    ns[:], sq[:].rearrange("p (h c) -> p h c", c=HD), axis=AX.X)
                # sqrt(sumsq/64) = |q|/8 for q; sqrt(sumsq) = |k| for k
                nc.scalar.activation(ns[:, 0:HL], ns[:, 0:HL], AF.Sqrt,
                                     scale=1.0 / 64.0)
                nc.scalar.activation(ns[:, HL : 2 * HL], ns[:, HL : 2 * HL],
                                     AF.Sqrt)
                inv = qsb.tile([128, 2 * HL], f32, name="inv")
                nc.vector.reciprocal(inv[:], ns[:])
                # qst = q * 8/|q| ; kst = k / |k|  (scale folded per head)
                qkst = qsb.tile([128, 2 * DL], bf16, name="qkst")
                nc.vector.tensor_tensor(
                    qkst[:].rearrange("p (h c) -> p h c", c=HD),
                    qk_sb[:].rearrange("p (h c) -> p h c", c=HD),
                    inv[:].rearrange("p (h o) -> p h o", o=1).broadcast_to(
                        (128, 2 * HL, HD)),
                    op=OP.mult)
                nc.sync.dma_start(qknat[128 * tt : 128 * (tt + 1), :], qkst[:])
                if tt % 8 == 7:
                    th = tt // 8
                    tsl = slice(1024 * th, 1024 * (th + 1))
                    for rb in range(2):
                        nc.sync.dma_start_transpose(
                            qT_sb[rb][:, tsl],
                            qknat[tsl, 128 * rb : 128 * (rb + 1)])
                        nc.sync.dma_start_transpose(
                            kT_sb[rb][:, tsl],
                            qknat[tsl, 256 + 128 * rb : 256 + 128 * (rb + 1)])

        # ---------------- attention + overlapped out-proj/RS ----------------
        with tc.tile_pool(name="scps", bufs=2, space="PSUM") as scps, \
             tc.tile_pool(name="pops", bufs=1, space="PSUM") as pops, \
             tc.tile_pool(name="ypps", bufs=2, space="PSUM") as ypps, \
             tc.tile_pool(name="exsb", bufs=3) as exsb, \
             tc.tile_pool(name="rssb", bufs=2) as rssb, \
             tc.tile_pool(name="ysb", bufs=8) as ysb:

            def rs_head(po, rsi, width):
                """evict po to SBUF (freeing its PSUM banks) and start the
                rowsum reciprocal on DVE (slow single-partition op, but fully
                off the critical path). Returns (posb, rinv)."""
                w2 = 2 * width
                posb = rssb.tile([65, 1024], f32, name="posb")
                nc.vector.tensor_copy(posb[:, 0:w2], po[0:65, 0:w2])
                rinv = rssb.tile([1, 1024], f32r, name="rinv")
                with nc.allow_low_precision(reason="f32r rowsum reciprocal"):
                    nc.vector.reciprocal(rinv[:, 0:w2], posb[64:65, 0:w2])
                return posb, rinv

            def rs_tail(posb, rinv, rb, tok0, width):
                """broadcast 1/rowsum via PE and normalize into aoT_sb[rb].
                Deferred via the piece queue until the reciprocal is done, so
                the bc matmul never waits in the PE queue."""
                for hh in range(2):
                    bc = ypps.tile([128, 512], f32, name="bc", tag="yp")
                    nc.tensor.matmul(
                        bc[0:64, 0:width], ones_sb[:],
                        rinv[:, width * hh : width * (hh + 1)],
                        start=True, stop=True)
                    bc_sb = rssb.tile([64, 512], f32, name="bc_sb")
                    nc.vector.tensor_copy(bc_sb[:, 0:width], bc[0:64, 0:width])
                    nc.vector.tensor_tensor(
                        aoT_sb[rb][64 * hh : 64 * (hh + 1), tok0 : tok0 + width],
                        posb[0:64, width * hh : width * (hh + 1)],
                        bc_sb[:, 0:width], op=OP.mult)

            def outproj_piece(ci, dt_):
                tok0c, wc = CHUNKS[ci]
                yp = ypps.tile([128, 512], f32, name="yp", tag="yp")
                for ft in range(2):
                    nc.tensor.matmul(
                        yp[:, 0:wc], WT_sb[:, ft, 128 * dt_ : 128 * (dt_ + 1)],
                        aoT_sb[ft][:, tok0c : tok0c + wc],
                        start=(ft == 0), stop=(ft == 1))
                yst = ysb.tile([128, 512], bf16, name="yst")
                nc.vector.tensor_copy(yst[:, 0:wc], yp[:, 0:wc])
                nc.sync.dma_start(
                    rs_in[ci][128 * dt_ : 128 * (dt_ + 1), :], yst[:, 0:wc])

            def outproj_finish(ci):
                nc.gpsimd.collective_compute(
                    "ReduceScatter", mybir.AluOpType.add,
                    replica_groups=[[0, 1, 2, 3], [4, 5, 6, 7]],
                    ins=[rs_in[ci].opt()], outs=[rs_out[ci].opt()])

            pieces = []
            git = 0  # global attention iteration counter
            for bi, (tok0, width) in enumerate(BLOCKS):
                for rb in range(2):
                    po = pops.tile([128, 2 * width], f32, name="po")
                    for j in range(NTT):
                        sc = scps.tile([128, 2 * width], f32, name="sc")
                        for hh in range(2):
                            nc.tensor.matmul(
                                sc[:, width * hh : width * (hh + 1)],
                                kT_sb[rb][64 * hh : 64 * (hh + 1),
                                          128 * j : 128 * (j + 1)],
                                qT_sb[rb][64 * hh : 64 * (hh + 1),
                                          tok0 : tok0 + width],
                                start=True, stop=True)
                        ex = exsb.tile([128, 2 * width], bf16, name="ex")
                        nc.scalar.activation(ex[:], sc[:], AF.Exp)
                        for hh in range(2):
                            nc.tensor.matmul(
                                po[0:65, width * hh : width * (hh + 1)],
                                v_sb[:, j, 2 * rb + hh, 0:65],
                                ex[:, width * hh : width * (hh + 1)],
                                start=(j == 0), stop=(j == NTT - 1))
                        # pop deferred pieces once their inputs (the slow
                        # rowsum reciprocal) are guaranteed done
                        if pieces and pieces[0][0] <= git:
                            pieces.pop(0)[1]()
                        git += 1
                    posb, rinv = rs_head(po, 2 * bi + rb, width)
                    pieces.append((git + 11,
                        lambda posb=posb, rinv=rinv, rb=rb, tok0=tok0:
                            rs_tail(posb, rinv, rb, tok0, width)))
                for ci in BLOCK_CHUNKS[bi]:
                    for dt_ in range(NDT):
                        pieces.append((git + 14,
                            lambda ci=ci, dt_=dt_: outproj_piece(ci, dt_)))
                    pieces.append((git + 14,
                        lambda ci=ci: outproj_finish(ci)))
            for _, p in pieces:
                p()
            # out DMAs batched at the end: all but the last chunk's CC are
            # long done, so these never head-of-line-block anything
            for ci, (tok0c, wc) in enumerate(CHUNKS):
                nc.sync.dma_start(out_ext[:, tok0c : tok0c + wc], rs_out[ci][:])

    nc.compile()
    return nc


@functools.lru_cache(maxsize=1)
def _get_nc():
    return _build_bass()


def _mp_normalize_rows(w):
    n = np.linalg.norm(w, axis=-1, keepdims=True)
    n = EPS + n * (1.0 / np.sqrt(w.shape[-1]))
    return (w / n) * (1.0 / np.sqrt(w.shape[-1]))


def make_in_maps(x, w_qkv, w_out):
    import ml_dtypes

    x = np.asarray(x, dtype=np.float32)
    w_qkv = np.asarray(w_qkv, dtype=np.float32)
    w_out = np.asarray(w_out, dtype=np.float32)

    wq_hat = _mp_normalize_rows(w_qkv)           # (3D, D) row-normalized/32
    wo_hat = _mp_normalize_rows(w_out)           # (D, D)
    woT = np.ascontiguousarray(wo_hat.T)         # (D_in, D_out)

    in_maps = []
    for c in range(NCORES):
        b, g = c // GROUP, c % GROUP
        rows = np.concatenate([
            np.arange(DL * g, DL * (g + 1)),
            D + np.arange(DL * g, DL * (g + 1)),
            2 * D + np.arange(DL * g, DL * (g + 1)),
        ])
        whT = np.ascontiguousarray(wq_hat[rows].T)   # (D, 768)
        in_maps.append({
            "xT": np.ascontiguousarray(x[b].T).astype(ml_dtypes.bfloat16),
            "whT": whT.astype(ml_dtypes.bfloat16),
            "WT": np.ascontiguousarray(
                woT[DL * g : DL * (g + 1)]).astype(ml_dtypes.bfloat16),
        })
    return in_maps


def kernel(x: np.ndarray, w_qkv: np.ndarray, w_out: np.ndarray) -> np.ndarray:
    from concourse.bass_utils import run_bass_kernel_spmd

    in_maps = make_in_maps(x, w_qkv, w_out)
    nc = _get_nc()
    res = run_bass_kernel_spmd(nc, in_maps, core_ids=list(range(NCORES)))

    out = np.empty((B, T, D), dtype=np.float32)
    for c in range(NCORES):
        b, g = c // GROUP, c % GROUP
        out[b][:, DL * g : DL * (g + 1)] = res.results[c]["out"].astype(np.float32).T
    return out


# revision 38
# speedup vs baseline: 1.0120x; 1.0120x over previous
"""Distributed Bass kernel for nn_Attention (B=2, T=2048, D=1024, H=16) on 8 TRN2 cores.

Sharding: core c -> (batch b = c//4, head-group g = c%4, heads 4g..4g+3).
QKV tensor-parallel over heads, out-proj row-parallel + ReduceScatter(4-rank groups).

v2: host-normalized weights, K=64 row-tiled score matmuls (two heads
concurrently on PE row-group halves), norms folded into q/k so the softmax
exp is a single unscaled [128,1024] ACT instruction per (head-pair, j),
bf16 output, chunked ReduceScatter overlapped with attention.
"""

import functools
import numpy as np
from contextlib import ExitStack

B, T, D, H, HD = 2, 2048, 1024, 16, 64
EPS = 1e-4
NCORES, GROUP = 8, 4
HL = H // GROUP          # heads per core = 4
DL = HL * HD             # local feature cols = 256
NTT = T // 128           # 16 token tiles
NDT = D // 128           # 8 d tiles
WCOLS = 3 * DL           # 768 qkv output cols per core

# attention q-blocks (tok0, width) and reduce-scatter chunks (tok0, width)
BLOCKS = [(0, 512), (512, 512), (1024, 512), (1536, 512)]
CHUNKS = [(0, 512), (512, 512), (1024, 512), (1536, 512)]
# chunk ids queued after each block completes
BLOCK_CHUNKS = [[0], [1], [2], [3]]


def _build_bass():
    import concourse.bass as bass
    import concourse.tile as tile
    from concourse import bacc, mybir

    f32 = mybir.dt.float32
    f32r = mybir.dt.float32r
    bf16 = mybir.dt.bfloat16
    AX = mybir.AxisListType
    OP = mybir.AluOpType
    AF = mybir.ActivationFunctionType

    nc = bacc.Bacc("TRN2", target_bir_lowering=False, debug=False, num_devices=NCORES)

    xT_ext = nc.dram_tensor("xT", [D, T], bf16, kind="ExternalInput").ap()
    whT_ext = nc.dram_tensor("whT", [D, WCOLS], bf16, kind="ExternalInput").ap()
    WT_ext = nc.dram_tensor("WT", [DL, D], bf16, kind="ExternalInput").ap()
    out_ext = nc.dram_tensor("out", [DL, T], bf16, kind="ExternalOutput").ap()

    ones_np = np.ones((1, 64), dtype=np.float32)

    with tile.TileContext(nc) as tc, ExitStack() as ctx:
        # ---------------- persistent pools ----------------
        pers = ctx.enter_context(tc.tile_pool(name="pers", bufs=1))
        dram = ctx.enter_context(tc.tile_pool(name="dram", bufs=1, space="DRAM"))

        ones_sb = pers.tile([1, 64], f32r)
        ones_dram = nc.inline_tensor(ones_np, name="ones_c")
        nc.gpsimd.dma_start(ones_sb[:], ones_dram.ap())

        xT_sb = pers.tile([128, NDT, T], bf16)
        whT_sb = pers.tile([128, NDT, WCOLS], bf16)
        WT_sb = pers.tile([128, 2, D], bf16)
        qT_sb = [pers.tile([128, T], bf16, name=f"qT{rb}") for rb in range(2)]
        kT_sb = [pers.tile([128, T], bf16, name=f"kT{rb}") for rb in range(2)]
        v_sb = pers.tile([128, NTT, HL, 65], bf16)
        aoT_sb = [pers.tile([128, T], bf16, name=f"aoT{rb}") for rb in range(2)]

        qknat = dram.tile([T, 2 * DL], bf16)
        rs_in = [dram.tile([D, w], bf16, name=f"rs_in{k}")
                 for k, (_, w) in enumerate(CHUNKS)]
        rs_out = [dram.tile([DL, w], bf16, name=f"rs_out{k}")
                  for k, (_, w) in enumerate(CHUNKS)]


        # ---------------- input DMAs (WT first: small, feeds PE warm-up) ----
        nc.sync.dma_start(WT_sb[:], WT_ext.rearrange("(n p) d -> p n d", p=128))
        nc.sync.dma_start(whT_sb[:], whT_ext.rearrange("(n p) c -> p n c", p=128))
        # x in 4 token chunks so QKV can start after the first lands
        for xc in range(4):
            nc.gpsimd.dma_start(
                xT_sb[:, :, 512 * xc : 512 * (xc + 1)],
                xT_ext.rearrange("(n p) t -> p n t", p=128)[
                    :, :, 512 * xc : 512 * (xc + 1)],
            )
        # ones column for the fused av-rowsum matmul
        nc.vector.memset(v_sb[:, :, :, 64:65], 1.0)

        # ---------------- QKV phase ----------------
        with tc.tile_pool(name="qps", bufs=2, space="PSUM") as qps, \
             tc.tile_pool(name="qsb", bufs=3) as qsb, \
             tc.tile_pool(name="wps", bufs=2, space="PSUM") as wps:
            # PE warm-up: dependency-free matmuls while x/whT stream in
            for wu in range(14):
                wt = wps.tile([128, 512], f32, name="wt", tag="warm")
                nc.tensor.matmul(wt[:], WT_sb[:, 0, 0:128], WT_sb[:, 0, 0:512],
                                 start=True, stop=True)

            for tt in range(NTT):
                ps = qps.tile([128, WCOLS], f32)
                for dt_ in range(NDT):
                    lhsT = xT_sb[:, dt_, 128 * tt : 128 * (tt + 1)]
                    nc.tensor.matmul(ps[:, 0:512], lhsT, whT_sb[:, dt_, 0:512],
                                     start=(dt_ == 0), stop=(dt_ == NDT - 1))
                    nc.tensor.matmul(ps[:, 512:768], lhsT, whT_sb[:, dt_, 512:768],
                                     start=(dt_ == 0), stop=(dt_ == NDT - 1))
                # evict q+k raw and v quickly so ps recycles (ACT, 2 instrs)
                qk_sb = qsb.tile([128, 2 * DL], bf16, name="qk_sb")
                nc.scalar.activation(qk_sb[:], ps[:, 0 : 2 * DL], AF.Copy)
                nc.scalar.activation(
                    v_sb[:, tt, :, 0:64],
                    ps[:, 2 * DL : 3 * DL].rearrange("p (h c) -> p h c", c=HD),
                    AF.Copy)
                # per-head norms of q and k off the SBUF copy
                sq = qsb.tile([128, 2 * DL], bf16, name="sq")
                nc.vector.tensor_tensor(sq[:], qk_sb[:], qk_sb[:], op=OP.mult)
                ns = qsb.tile([128, 2 * HL], f32, name="ns")
                nc.vector.reduce_sum(
                    ns[:], sq[:].rearrange("p (h c) -> p h c", c=HD), axis=AX.X)
                # sqrt(sumsq/64) = |q|/8 for q; sqrt(sumsq) = |k| for k
                nc.scalar.activation(ns[:, 0:HL], ns[:, 0:HL], AF.Sqrt,
                                     scale=1.0 / 64.0)
                nc.scalar.activation(ns[:, HL : 2 * HL], ns[:, HL : 2 * HL],
                                     AF.Sqrt)
                inv = qsb.tile([128, 2 * HL], f32, name="inv")
                nc.vector.reciprocal(inv[:], ns[:])
                # qst = q * 8/|q| ; kst = k / |k|  (scale folded per head)
                qkst = qsb.tile([128, 2 * DL], bf16, name="qkst")
                nc.vector.tensor_tensor(
                    qkst[:].rearrange("p (h c) -> p h c", c=HD),
                    qk_sb[:].rearrange("p (h c) -> p h c", c=HD),
                    inv[:].rearrange("p (h o) -> p h o", o=1).broadcast_to(
                        (128, 2 * HL, HD)),
                    op=OP.mult)
                nc.sync.dma_start(qknat[128 * tt : 128 * (tt + 1), :], qkst[:])
                if tt % 8 == 7:
                    th = tt // 8
                    tsl = slice(1024 * th, 1024 * (th + 1))
                    for rb in range(2):
                        nc.sync.dma_start_transpose(
                            qT_sb[rb][:, tsl],
                            qknat[tsl, 128 * rb : 128 * (rb + 1)])
                        nc.sync.dma_start_transpose(
                            kT_sb[rb][:, tsl],
                            qknat[tsl, 256 + 128 * rb : 256 + 128 * (rb + 1)])

        # ---------------- attention + overlapped out-proj/RS ----------------
        with tc.tile_pool(name="scps", bufs=2, space="PSUM") as scps, \
             tc.tile_pool(name="pops", bufs=1, space="PSUM") as pops, \
             tc.tile_pool(name="ypps", bufs=2, space="PSUM") as ypps, \
             tc.tile_pool(name="exsb", bufs=3) as exsb, \
             tc.tile_pool(name="rssb", bufs=2) as rssb, \
             tc.tile_pool(name="ysb", bufs=8) as ysb, \
             tc.tile_pool(name="sumsb", bufs=2) as sumsb:

            def rs_head(po, width):
                """evict po to SBUF (freeing its PSUM banks) and start the
                rowsum reciprocal on DVE (slow single-partition op, but fully
                off the critical path). Emitted as a deferred piece so the
                copy never camps on the DVE queue head. Returns (posb, rinv)."""
                w2 = 2 * width
                posb = rssb.tile([65, 1024], f32, name="posb")
                nc.vector.tensor_copy(posb[:, 0:w2], po[0:65, 0:w2])
                rinv = rssb.tile([1, 1024], f32r, name="rinv")
                with nc.allow_low_precision(reason="f32r rowsum reciprocal"):
                    nc.vector.reciprocal(rinv[:, 0:w2], posb[64:65, 0:w2])
                return posb, rinv

            def rs_tail(posb, rinv, rb, tok0, width):
                """broadcast 1/rowsum via PE and normalize into aoT_sb[rb].
                Deferred via the piece queue until the reciprocal is done, so
                the bc matmul never waits in the PE queue."""
                for hh in range(2):
                    bc = ypps.tile([128, 512], f32, name="bc", tag="yp")
                    nc.tensor.matmul(
                        bc[0:64, 0:width], ones_sb[:],
                        rinv[:, width * hh : width * (hh + 1)],
                        start=True, stop=True)
                    bc_sb = rssb.tile([64, 512], f32, name="bc_sb")
                    nc.vector.tensor_copy(bc_sb[:, 0:width], bc[0:64, 0:width])
                    nc.vector.tensor_tensor(
                        aoT_sb[rb][64 * hh : 64 * (hh + 1), tok0 : tok0 + width],
                        posb[0:64, width * hh : width * (hh + 1)],
                        bc_sb[:, 0:width], op=OP.mult)

            def outproj_piece(ci, dt_):
                tok0c, wc = CHUNKS[ci]
                yp = ypps.tile([128, 512], f32, name="yp", tag="yp")
                for ft in range(2):
                    nc.tensor.matmul(
                        yp[:, 0:wc], WT_sb[:, ft, 128 * dt_ : 128 * (dt_ + 1)],
                        aoT_sb[ft][:, tok0c : tok0c + wc],
                        start=(ft == 0), stop=(ft == 1))
                yst = ysb.tile([128, 512], bf16, name="yst")
                nc.vector.tensor_copy(yst[:, 0:wc], yp[:, 0:wc])
                nc.sync.dma_start(
                    rs_in[ci][128 * dt_ : 128 * (dt_ + 1), :], yst[:, 0:wc])

            def outproj_finish(ci):
                nc.gpsimd.collective_compute(
                    "ReduceScatter", mybir.AluOpType.add,
                    replica_groups=[[0, 1, 2, 3], [4, 5, 6, 7]],
                    ins=[rs_in[ci].opt()], outs=[rs_out[ci].opt()])

            pieces = []
            git = 0  # global attention iteration counter
            for bi, (tok0, width) in enumerate(BLOCKS):
                for rb in range(2):
                    po = pops.tile([128, 2 * width], f32, name="po")
                    for j in range(NTT):
                        sc = scps.tile([128, 2 * width], f32, name="sc")
                        for hh in range(2):
                            nc.tensor.matmul(
                                sc[:, width * hh : width * (hh + 1)],
                                kT_sb[rb][64 * hh : 64 * (hh + 1),
                                          128 * j : 128 * (j + 1)],
                                qT_sb[rb][64 * hh : 64 * (hh + 1),
                                          tok0 : tok0 + width],
                                start=True, stop=True)
                        ex = exsb.tile([128, 2 * width], bf16, name="ex")
                        nc.scalar.activation(ex[:], sc[:], AF.Exp)
                        for hh in range(2):
                            nc.tensor.matmul(
                                po[0:65, width * hh : width * (hh + 1)],
                                v_sb[:, j, 2 * rb + hh, 0:65],
                                ex[:, width * hh : width * (hh + 1)],
                                start=(j == 0), stop=(j == NTT - 1))
                        # pop the first READY deferred piece (skip entries
                        # whose inputs, e.g. the rowsum reciprocal, are not
                        # guaranteed done yet)
                        for pi in range(len(pieces)):
                            if pieces[pi][0] <= git:
                                pieces.pop(pi)[1]()
                                break
                        git += 1
                    hold = {}
                    pieces.append((git + 1,
                        lambda po=po, hold=hold:
                            hold.update(pr=rs_head(po, width))))
                    pieces.append((git + 11,
                        lambda hold=hold, rb=rb, tok0=tok0:
                            rs_tail(hold["pr"][0], hold["pr"][1], rb, tok0,
                                    width)))
                for ci in BLOCK_CHUNKS[bi]:
                    for dt_ in range(NDT):
                        pieces.append((git + 14,
                            lambda ci=ci, dt_=dt_: outproj_piece(ci, dt_)))
                    pieces.append((git + 14,
                        lambda ci=ci: outproj_finish(ci)))
            for _, p in pieces:
                p()
            # out DMAs batched at the end: all but the last chunk's CC are
            # long done, so these never head-of-line-block anything
            for ci, (tok0c, wc) in enumerate(CHUNKS):
                nc.sync.dma_start(out_ext[:, tok0c : tok0c + wc], rs_out[ci][:])

    nc.compile()
    return nc


@functools.lru_cache(maxsize=1)
def _get_nc():
    return _build_bass()


def _mp_normalize_rows(w):
    n = np.linalg.norm(w, axis=-1, keepdims=True)
    n = EPS + n * (1.0 / np.sqrt(w.shape[-1]))
    return (w / n) * (1.0 / np.sqrt(w.shape[-1]))


def make_in_maps(x, w_qkv, w_out):
    import ml_dtypes

    x = np.asarray(x, dtype=np.float32)
    w_qkv = np.asarray(w_qkv, dtype=np.float32)
    w_out = np.asarray(w_out, dtype=np.float32)

    wq_hat = _mp_normalize_rows(w_qkv)           # (3D, D) row-normalized/32
    wo_hat = _mp_normalize_rows(w_out)           # (D, D)
    woT = np.ascontiguousarray(wo_hat.T)         # (D_in, D_out)

    in_maps = []
    for c in range(NCORES):
        b, g = c // GROUP, c % GROUP
        rows = np.concatenate([
            np.arange(DL * g, DL * (g + 1)),
            D + np.arange(DL * g, DL * (g + 1)),
            2 * D + np.arange(DL * g, DL * (g + 1)),
        ])
        whT = np.ascontiguousarray(wq_hat[rows].T)   # (D, 768)
        in_maps.append({
            "xT": np.ascontiguousarray(x[b].T).astype(ml_dtypes.bfloat16),
            "whT": whT.astype(ml_dtypes.bfloat16),
            "WT": np.ascontiguousarray(
                woT[DL * g : DL * (g + 1)]).astype(ml_dtypes.bfloat16),
        })
    return in_maps


def kernel(x: np.ndarray, w_qkv: np.ndarray, w_out: np.ndarray) -> np.ndarray:
    from concourse.bass_utils import run_bass_kernel_spmd

    in_maps = make_in_maps(x, w_qkv, w_out)
    nc = _get_nc()
    res = run_bass_kernel_spmd(nc, in_maps, core_ids=list(range(NCORES)))

    out = np.empty((B, T, D), dtype=np.float32)
    for c in range(NCORES):
        b, g = c // GROUP, c % GROUP
        out[b][:, DL * g : DL * (g + 1)] = res.results[c]["out"].astype(np.float32).T
    return out
